# revision 13
# baseline (speedup 1.0000x reference)
"""DCGRU cell on 8 Trainium2 NeuronCores.

Strategy (data-parallel over batch B=64 -> 8 per core):
  - Sparse supports are densified on host into S^T [2048, 2048] fp32 and
    streamed tile-by-tile from HBM as matmul stationary operands; the
    diffusion spmm runs as dense fp32r matmul on the PE at 1 cycle/row.
  - Activations live node-major ("natural") [n, (b,f)]; the Chebyshev
    recursion x2 = 2 S x1 - x0 is folded as xs2' = S x1 - 0.5 x0 with
    W2' = 2 W2 (host pre-scales W rows of the k=2 terms).
  - The per-matrix projection contraction (over features f and matrix
    index m) needs feature-major operands, so each diffusion output is
    PE-transposed into bf16 tiles xs^T. W is host-reordered so state
    features contract as K=64 groups (W rows duplicated at partition
    base 64 so lhsT/rhs bases match) and the 5x2 input features as one
    K=10 group gathered into partition-base-aligned packed tiles.
  - Gate output stays feature-major: r is transposed back and multiplied
    into the natural x0 in place (building the candidate input); u and c
    take a DRAM round-trip; the final GRU combine runs in natural layout
    and the output is written as [n, (b, u)] which the host untransposes.
"""

import numpy as np

import concourse.bass as bass
from concourse import bacc
import concourse.mybir as mybir
import concourse.tile as tile
from concourse.bass_utils import run_bass_kernel_spmd
from concourse.masks import make_identity

N = 2048            # nodes
B = 64              # global batch
BL = 8              # batch per core
NCORES = 8
D_IN = 2
U = 64              # hidden units
M = 5               # 1 + 2 supports * K
F = D_IN + U        # 66
NB = N // 128       # 16 node blocks
SC = BL * U         # 512 state cols in natural layout
IC = BL * D_IN      # 16 input cols
CW = SC + IC        # 528 total natural cols
PK = M * D_IN       # 10 packed input rows per batch

F32 = mybir.dt.float32
F32R = mybir.dt.float32r
BF16 = mybir.dt.bfloat16


def _r(ap):
    return ap.bitcast(F32R)


def _build_nc():
    nc = bacc.Bacc(None, target_bir_lowering=False)

    x0d = nc.declare_dram_parameter("x0", [N, CW], F32R, isOutput=False)
    satd = nc.declare_dram_parameter("sat", [N, N], F32R, isOutput=False)
    sbtd = nc.declare_dram_parameter("sbt", [N, N], F32R, isOutput=False)
    wgsd = nc.declare_dram_parameter("wgs", [128, M * 128], BF16, isOutput=False)
    wgid = nc.declare_dram_parameter("wgi", [128, 128], BF16, isOutput=False)
    wcsd = nc.declare_dram_parameter("wcs", [128, M * U], BF16, isOutput=False)
    wcid = nc.declare_dram_parameter("wci", [128, U], BF16, isOutput=False)
    bgd = nc.declare_dram_parameter("bg", [128, 1], F32, isOutput=False)
    bcd = nc.declare_dram_parameter("bc", [U, 1], F32, isOutput=False)
    outd = nc.declare_dram_parameter("out", [N, SC], F32, isOutput=True)
    ubufd = nc.dram_tensor("ubuf", [128, BL * 1024], F32)
    cbufd = nc.dram_tensor("cbuf", [128, BL * 1024], F32)

    with tile.TileContext(nc) as tc:
        _emit(nc, tc, x0d, satd, sbtd, wgsd, wgid, wcsd, wcid, bgd, bcd,
              outd, ubufd, cbufd)
    nc.compile()
    return nc


def _emit(nc, tc, x0d, satd, sbtd, wgsd, wgid, wcsd, wcid, bgd, bcd, outd,
          ubufd, cbufd):
    from contextlib import ExitStack
    ctx = ExitStack()
    with ctx:
        consts = ctx.enter_context(tc.tile_pool(name="consts", bufs=1))
        nat = ctx.enter_context(tc.tile_pool(name="nat", bufs=1))
        xst = ctx.enter_context(tc.tile_pool(name="xst", bufs=1))
        x2p = ctx.enter_context(tc.tile_pool(name="x2p", bufs=3))
        spool = ctx.enter_context(tc.tile_pool(name="spool", bufs=8))
        small = ctx.enter_context(tc.tile_pool(name="small", bufs=3))
        psum = ctx.enter_context(tc.tile_pool(name="psum", bufs=8, space="PSUM"))

        ident = consts.tile([128, 128], F32)
        make_identity(nc, ident[:])

        wgs = consts.tile([128, M * 128], BF16)
        wgi = consts.tile([128, 128], BF16)
        wcs = consts.tile([128, M * U], BF16)
        wci = consts.tile([128, U], BF16)
        bg = consts.tile([128, 1], F32)
        bc = consts.tile([U, 1], F32)
        for dst, src in ((wgs, wgsd), (wgi, wgid), (wcs, wcsd), (wci, wcid),
                         (bg, bgd), (bc, bcd)):
            nc.sync.dma_start(dst[:], src[:])

        # natural-layout activations: [128, NB * CW], block i at cols i*CW
        x0sb = nat.tile([128, NB * CW], F32R, tag="x0")
        x1sb = nat.tile([128, NB * CW], F32R, tag="x1")
        x0dv = x0d.rearrange("(t p) c -> t p c", p=128)
        for i in range(NB):
            nc.sync.dma_start(x0sb[:, i * CW:(i + 1) * CW], x0dv[i])

        # xs^T state parts, bf16: block (m, j) = batches {2j, 2j+1}, all n
        xsts = xst.tile([128, M * 4 * N], BF16, tag="xsts")
        # input-feature diffusion, natural gather: cols b*32 + m*D_IN + fi
        xicat = xst.tile([128, NB * 256], BF16, tag="xicat")
        nc.vector.memset(xicat[:], 0.0)
        # packed input-feature rhs: tile t, batch b=3t+k at rows 32*k
        xpk = [xst.tile([128, N], BF16, tag=f"xpk{t}", name=f"xpk{t}")
               for t in range(3)]

        def xst_s(m, j):
            return xsts[:, (m * 4 + j) * N:(m * 4 + j + 1) * N]

        def transpose_to_xst(m, i, src_ap, with_input):
            """src_ap: natural block [128, CW-ish]; writes xs^T tiles."""
            for j in range(4):
                pt = psum.tile([128, 128], F32, tag="ps")
                nc.tensor.transpose(
                    pt[:], src_ap[:, j * 128:(j + 1) * 128].bitcast(F32),
                    ident[:])
                nc.vector.tensor_copy(
                    xst_s(m, j)[:, i * 128:(i + 1) * 128], pt[:])
            if with_input:
                # gather input cols into xicat (b,f)-strided -> (b,m,f)
                src3 = src_ap[:, SC:SC + IC].bitcast(F32).rearrange(
                    "p (b f) -> p b f", b=BL)
                dst3 = xicat[:, i * 256:(i + 1) * 256].rearrange(
                    "p (b r) -> p b r", r=32)[:, :, m * D_IN:(m + 1) * D_IN]
                nc.vector.tensor_copy(dst3, src3)

        def spmm(std, xsrc, chunks, dest_cb):
            """Y = S @ X. std: DRAM S^T; xsrc: natural tile [128, NB*CW];
            dest_cb(i, psum_list) consumes each output row-block."""
            for i in range(NB):
                pts = [psum.tile([128, c1 - c0], F32, tag="ps", name=f"pmm{i}_{c0}")
                       for (c0, c1) in chunks]
                for j in range(NB):
                    st = spool.tile([128, 128], F32R, tag="st")
                    nc.sync.dma_start(
                        st[:], std[j * 128:(j + 1) * 128, i * 128:(i + 1) * 128])
                    for ci, (c0, c1) in enumerate(chunks):
                        nc.tensor.matmul(
                            pts[ci][:], st[:],
                            xsrc[:, j * CW + c0:j * CW + c1],
                            start=(j == 0), stop=(j == NB - 1))
                dest_cb(i, pts)

        GCH = [(0, 264), (264, 528)]
        CCH = [(0, 256), (256, 512)]

        def dconv(xnat, x1nat, is_gate):
            """Emit one diffusion-conv's spmm + transpose stages."""
            chunks = GCH if is_gate else CCH
            wid = SC + (IC if is_gate else 0)
            for i in range(NB):
                transpose_to_xst(0, i, xnat[:, i * CW:i * CW + CW], is_gate)
            for sup, std in ((0, satd), (1, sbtd)):
                m1, m2 = 1 + 2 * sup, 2 + 2 * sup

                def x1_sink(i, pts):
                    for pt, (c0, c1) in zip(pts, chunks):
                        nc.vector.tensor_copy(
                            x1nat[:, i * CW + c0:i * CW + c1], pt[:])
                    transpose_to_xst(m1, i, x1nat[:, i * CW:i * CW + CW],
                                     is_gate)

                spmm(std, xnat, chunks, x1_sink)

                # x2' = S x1 - 0.5 x0  (W of the k=2 terms pre-doubled)
                def x2_sink(i, pts):
                    blk = x2p.tile([128, CW], F32R, tag="x2")
                    for pt, (c0, c1) in zip(pts, chunks):
                        nc.vector.scalar_tensor_tensor(
                            blk[:, c0:c1],
                            xnat[:, i * CW + c0:i * CW + c1].bitcast(F32),
                            -0.5, pt[:],
                            mybir.AluOpType.mult, mybir.AluOpType.add)
                    transpose_to_xst(m2, i, blk[:, 0:wid], is_gate)

                spmm(std, x1nat, chunks, x2_sink)

        identb = consts.tile([128, 128], BF16)

        def finalize_inputs2():
            nc.vector.tensor_copy(identb[:], ident[:])
            for i in range(NB):
                for t in range(3):
                    w = 96 if t < 2 else 64
                    pt = psum.tile([w, 128], BF16, tag="ps", name=f"pfin{t}")
                    nc.tensor.transpose(
                        pt[:], xicat[:, i * 256 + t * 96:i * 256 + t * 96 + w],
                        identb[:])
                    nc.vector.tensor_copy(xpk[t][:w, i * 128:(i + 1) * 128],
                                          pt[:])

        def w_stage(is_gate):
            """Projection + activation. Gate: sigmoid -> r (into x0sb),
            u (to DRAM). Cand: tanh -> c (to DRAM)."""
            ws, wi, O = (wgs, wgi, 128) if is_gate else (wcs, wci, U)
            for b in range(BL):
                t, bl = b // 3, b % 3
                for c in range(4):  # n-chunks of 512
                    pt = psum.tile([O, 512], F32, tag="ps")
                    bp = (b % 2) * U
                    for m in range(M):
                        rs = xst_s(m, b // 2)[bp:bp + U, c * 512:(c + 1) * 512]
                        nc.tensor.matmul(pt[:], ws[bp:bp + U, m * O:(m + 1) * O],
                                         rs, start=(m == 0), stop=False)
                    ri = xpk[t][32 * bl:32 * bl + PK, c * 512:(c + 1) * 512]
                    nc.tensor.matmul(pt[:], wi[32 * bl:32 * bl + PK, :O], ri,
                                     start=False, stop=True)
                    h = c // 2
                    cols = slice(b * 1024 + 512 * (c % 2),
                                 b * 1024 + 512 * (c % 2) + 512)
                    if is_gate:
                        rb = small.tile([U, 512], F32, tag="rb")
                        nc.scalar.activation(rb[:], pt[:U, :],
                                             mybir.ActivationFunctionType.Sigmoid,
                                             bias=bg[:U, :])
                        ub = small.tile([U, 512], F32, tag="ub")
                        nc.scalar.activation(ub[:], pt[U:128, :],
                                             mybir.ActivationFunctionType.Sigmoid,
                                             bias=bg[U:128, :])
                        nc.sync.dma_start(ubufd[64 * h:64 * h + 64, cols], ub[:])
                        # r^T into x0 state cols (candidate input, in place)
                        for j in range(4):
                            i = 4 * c + j
                            rpt = psum.tile([128, U], F32, tag="ps")
                            nc.tensor.transpose(
                                rpt[:], rb[:, j * 128:(j + 1) * 128],
                                ident[:U, :U])
                            xsl = x0sb[:, i * CW + b * U:i * CW + (b + 1) * U]
                            nc.vector.tensor_mul(xsl, xsl.bitcast(F32), rpt[:])
                    else:
                        cb = small.tile([U, 512], F32, tag="cb")
                        nc.scalar.activation(cb[:], pt[:, :],
                                             mybir.ActivationFunctionType.Tanh,
                                             bias=bc[:])
                        nc.sync.dma_start(cbufd[64 * h:64 * h + 64, cols], cb[:])

        def final():
            """new_state = c + u*(state - c), natural layout, DMA out."""
            for b in range(BL):
                for i in range(NB):
                    h = i // 8
                    cols = slice(b * 1024 + 128 * (i % 8),
                                 b * 1024 + 128 * (i % 8) + 128)
                    ut = small.tile([U, 128], F32, tag="ut")
                    nc.sync.dma_start(ut[:], ubufd[64 * h:64 * h + 64, cols])
                    ct = small.tile([U, 128], F32, tag="ct")
                    nc.sync.dma_start(ct[:], cbufd[64 * h:64 * h + 64, cols])
                    stt = small.tile([128, U], F32R, tag="stt")
                    nc.sync.dma_start(
                        stt[:], x0d[i * 128:(i + 1) * 128, b * U:(b + 1) * U])
                    cpt = psum.tile([128, U], F32, tag="ps")
                    nc.tensor.transpose(cpt[:], ct[:], ident[:U, :U])
                    upt = psum.tile([128, U], F32, tag="ps")
                    nc.tensor.transpose(upt[:], ut[:], ident[:U, :U])
                    t1 = small.tile([128, U], F32, tag="t1")
                    nc.vector.tensor_sub(t1[:], stt[:].bitcast(F32), cpt[:])
                    t2 = small.tile([128, U], F32, tag="t2")
                    nc.vector.tensor_mul(t2[:], t1[:], upt[:])
                    ot = small.tile([128, U], F32, tag="ot")
                    nc.vector.tensor_add(ot[:], t2[:], cpt[:])
                    nc.sync.dma_start(
                        outd[i * 128:(i + 1) * 128, b * U:(b + 1) * U], ot[:])

        # ---- gate dconv ----
        dconv(x0sb, x1sb, True)
        finalize_inputs2()
        w_stage(True)
        # ---- candidate dconv (x0sb is now candX in its state cols) ----
        dconv(x0sb, x1sb, False)
        w_stage(False)
        final()


_NC_CACHE = {}


def _get_nc():
    if "nc" not in _NC_CACHE:
        _NC_CACHE["nc"] = _build_nc()
    return _NC_CACHE["nc"]


def _host_prep(inputs, state, edges1, vals1, edges2, vals2, W_gate, b_gate,
               W_cand, b_cand):
    import ml_dtypes
    inputs = np.asarray(inputs, np.float32)
    state = np.asarray(state, np.float32)

    def densify_T(edges, vals):
        ST = np.zeros((N, N), np.float32)
        np.add.at(ST, (np.asarray(edges[1]).astype(np.int64),
                       np.asarray(edges[0]).astype(np.int64)),
                  np.asarray(vals, np.float32))
        return ST

    SaT = densify_T(edges1, vals1)
    SbT = densify_T(edges2, vals2)

    def reorder(Wmat):
        Wmat = np.asarray(Wmat, np.float32)
        O = Wmat.shape[1]
        Wm = Wmat.reshape(F, M, O).copy()
        Wm[:, 2, :] *= 2.0
        Wm[:, 4, :] *= 2.0
        # state rows duplicated at partition bases 0 and 64
        Ws = np.ascontiguousarray(Wm[D_IN:].reshape(U, M * O))
        Ws2 = np.concatenate([Ws, Ws], 0)                       # [128, M*O]
        # input rows (m, fi) packed [10, O], replicated at bases 0/32/64
        Wi = np.ascontiguousarray(Wm[:D_IN].transpose(1, 0, 2).reshape(PK, O))
        Wi2 = np.zeros((128, O), np.float32)
        for base in (0, 32, 64):
            Wi2[base:base + PK] = Wi
        return (Ws2.astype(ml_dtypes.bfloat16), Wi2.astype(ml_dtypes.bfloat16))

    wgs, wgi = reorder(W_gate)
    wcs, wci = reorder(W_cand)
    bg = np.asarray(b_gate, np.float32).reshape(128, 1)
    bc = np.asarray(b_cand, np.float32).reshape(U, 1)

    in_maps = []
    for c in range(NCORES):
        bsl = slice(c * BL, (c + 1) * BL)
        st_c = state[bsl].reshape(BL, N, U)
        in_c = inputs[bsl].reshape(BL, N, D_IN)
        x0 = np.empty((N, CW), np.float32)
        x0[:, :SC] = st_c.transpose(1, 0, 2).reshape(N, SC)
        x0[:, SC:] = in_c.transpose(1, 0, 2).reshape(N, IC)
        in_maps.append(dict(x0=x0, sat=SaT, sbt=SbT, wgs=wgs, wgi=wgi,
                            wcs=wcs, wci=wci, bg=bg, bc=bc))
    return in_maps


def kernel(**inputs):
    nc = _get_nc()
    in_maps = _host_prep(**inputs)
    res = run_bass_kernel_spmd(nc, in_maps, list(range(NCORES)))
    outs = []
    for c in range(NCORES):
        o = np.asarray(res.results[c]["out"])          # [N, (b, u)]
        outs.append(o.reshape(N, BL, U).transpose(1, 0, 2).reshape(BL, N * U))
    return np.concatenate(outs, 0).astype(np.float32)


# revision 15
# speedup vs baseline: 2.1535x; 2.1535x over previous
"""DCGRU cell on 8 Trainium2 NeuronCores.

Strategy (data-parallel over batch B=64 -> 8 per core):
  - Sparse supports are densified on host into S^T [2048, 2048] bf16 and
    streamed column-batched from HBM as matmul stationary operands; the
    diffusion spmm runs as dense bf16 matmul (fp32 PSUM accumulate).
  - Activations live node-major ("natural") [n, (b,f)] in bf16; the
    Chebyshev recursion x2 = 2 S x1 - x0 is folded as xs2' = S x1 -
    0.5 x0 with W2' = 2 W2 (host pre-scales the k=2 W rows).
  - The projection contraction (over features f and matrix index m)
    needs feature-major operands, so each diffusion output is
    PE-transposed into bf16 tiles xs^T. W is host-reordered so state
    features contract as K=64 groups (W rows duplicated at partition
    base 64 so lhsT/rhs bases match) and the 5x2 input features as one
    K=10 group gathered into partition-base-aligned packed tiles.
  - Gate output stays feature-major: r is transposed back and multiplied
    into the natural x0 in place (building the candidate input); u and c
    take a DRAM round-trip; the final GRU combine runs in natural layout
    against an fp32 state re-read, and the output is written as
    [n, (b, u)] fp32 which the host untransposes.
"""

import numpy as np

import concourse.bass as bass
from concourse import bacc
import concourse.mybir as mybir
import concourse.tile as tile
from concourse.bass_utils import run_bass_kernel_spmd
from concourse.masks import make_identity

N = 2048            # nodes
B = 64              # global batch
BL = 8              # batch per core
NCORES = 8
D_IN = 2
U = 64              # hidden units
M = 5               # 1 + 2 supports * K
F = D_IN + U        # 66
NB = N // 128       # 16 node blocks
SC = BL * U         # 512 state cols in natural layout
IC = BL * D_IN      # 16 input cols
CW = SC + IC        # 528 total natural cols
PK = M * D_IN       # 10 packed input rows per batch

F32 = mybir.dt.float32
BF16 = mybir.dt.bfloat16


def _build_nc():
    nc = bacc.Bacc(None, target_bir_lowering=False)

    x0d = nc.declare_dram_parameter("x0", [N, CW], BF16, isOutput=False)
    stfd = nc.declare_dram_parameter("statef", [N, SC], F32, isOutput=False)
    satd = nc.declare_dram_parameter("sat", [N, N], BF16, isOutput=False)
    sbtd = nc.declare_dram_parameter("sbt", [N, N], BF16, isOutput=False)
    wgsd = nc.declare_dram_parameter("wgs", [128, M * 128], BF16, isOutput=False)
    wgid = nc.declare_dram_parameter("wgi", [128, 128], BF16, isOutput=False)
    wcsd = nc.declare_dram_parameter("wcs", [128, M * U], BF16, isOutput=False)
    wcid = nc.declare_dram_parameter("wci", [128, U], BF16, isOutput=False)
    bgd = nc.declare_dram_parameter("bg", [128, 1], F32, isOutput=False)
    bcd = nc.declare_dram_parameter("bc", [U, 1], F32, isOutput=False)
    outd = nc.declare_dram_parameter("out", [N, SC], F32, isOutput=True)
    ubufd = nc.dram_tensor("ubuf", [128, BL * 1024], F32)
    cbufd = nc.dram_tensor("cbuf", [128, BL * 1024], F32)

    with tile.TileContext(nc) as tc:
        _emit(nc, tc, x0d, stfd, satd, sbtd, wgsd, wgid, wcsd, wcid, bgd,
              bcd, outd, ubufd, cbufd)
    nc.compile()
    return nc


def _emit(nc, tc, x0d, stfd, satd, sbtd, wgsd, wgid, wcsd, wcid, bgd, bcd,
          outd, ubufd, cbufd):
    from contextlib import ExitStack
    ctx = ExitStack()
    with ctx:
        consts = ctx.enter_context(tc.tile_pool(name="consts", bufs=1))
        nat = ctx.enter_context(tc.tile_pool(name="nat", bufs=1))
        xst = ctx.enter_context(tc.tile_pool(name="xst", bufs=1))
        x2p = ctx.enter_context(tc.tile_pool(name="x2p", bufs=3))
        spool = ctx.enter_context(tc.tile_pool(name="spool", bufs=3))
        small = ctx.enter_context(tc.tile_pool(name="small", bufs=2))
        psum = ctx.enter_context(tc.tile_pool(name="psum", bufs=8, space="PSUM"))

        ident = consts.tile([128, 128], F32)
        make_identity(nc, ident[:])
        identb = consts.tile([128, 128], BF16)
        nc.vector.tensor_copy(identb[:], ident[:])

        wgs = consts.tile([128, M * 128], BF16)
        wgi = consts.tile([128, 128], BF16)
        wcs = consts.tile([128, M * U], BF16)
        wci = consts.tile([128, U], BF16)
        bg = consts.tile([128, 1], F32)
        bc = consts.tile([U, 1], F32)
        for dst, src in ((wgs, wgsd), (wgi, wgid), (wcs, wcsd), (wci, wcid),
                         (bg, bgd), (bc, bcd)):
            nc.sync.dma_start(dst[:], src[:])

        # natural-layout activations (bf16): block i at cols i*CW
        x0sb = nat.tile([128, NB * CW], BF16, tag="x0")
        x1sb = nat.tile([128, NB * CW], BF16, tag="x1")
        x0dv = x0d.rearrange("(t p) c -> t p c", p=128)
        for i in range(NB):
            nc.sync.dma_start(x0sb[:, i * CW:(i + 1) * CW], x0dv[i])

        # xs^T state parts, bf16: block (m, j) = batches {2j, 2j+1}, all n
        xsts = xst.tile([128, M * 4 * N], BF16, tag="xsts")
        # input-feature diffusion, natural gather: cols b*32 + m*D_IN + fi
        xicat = xst.tile([128, NB * 256], BF16, tag="xicat")
        nc.vector.memset(xicat[:], 0.0)
        # packed input-feature rhs: tile t, batch b=3t+k at rows 32*k
        xpk = [xst.tile([128, N], BF16, tag=f"xpk{t}", name=f"xpk{t}")
               for t in range(3)]

        def xst_s(m, j):
            return xsts[:, (m * 4 + j) * N:(m * 4 + j + 1) * N]

        def transpose_to_xst(m, i, src_ap, with_input):
            """src_ap: natural bf16 block [128, CW-ish]; writes xs^T."""
            for j in range(4):
                pt = psum.tile([128, 128], BF16, tag="ps")
                nc.tensor.transpose(
                    pt[:], src_ap[:, j * 128:(j + 1) * 128], identb[:])
                nc.vector.tensor_copy(
                    xst_s(m, j)[:, i * 128:(i + 1) * 128], pt[:])
            if with_input:
                # gather input cols into xicat (b,f)-strided -> (b,m,f)
                src3 = src_ap[:, SC:SC + IC].rearrange("p (b f) -> p b f", b=BL)
                dst3 = xicat[:, i * 256:(i + 1) * 256].rearrange(
                    "p (b r) -> p b r", r=32)[:, :, m * D_IN:(m + 1) * D_IN]
                nc.vector.tensor_copy(dst3, src3)

        def spmm(std, xsrc, chunks, dest_cb):
            """Y = S @ X (bf16). Per row-block: one column-batched S DMA,
            then K-accumulated matmuls; dest_cb(i, psum_list) consumes."""
            stdv = std.rearrange("(j p) c -> p j c", p=128)
            for i in range(NB):
                sc = spool.tile([128, NB * 128], BF16, tag="sc")
                nc.sync.dma_start(
                    sc[:].rearrange("p (j c) -> p j c", j=NB),
                    stdv[:, :, i * 128:(i + 1) * 128])
                pts = [psum.tile([128, c1 - c0], F32, tag="ps",
                                 name=f"pmm{i}_{c0}")
                       for (c0, c1) in chunks]
                for j in range(NB):
                    for ci, (c0, c1) in enumerate(chunks):
                        nc.tensor.matmul(
                            pts[ci][:], sc[:, j * 128:(j + 1) * 128],
                            xsrc[:, j * CW + c0:j * CW + c1],
                            start=(j == 0), stop=(j == NB - 1))
                dest_cb(i, pts)

        GCH = [(0, 512), (512, 528)]
        CCH = [(0, 512)]

        def dconv(xnat, x1nat, is_gate):
            """Emit one diffusion-conv's spmm + transpose stages."""
            chunks = GCH if is_gate else CCH
            wid = SC + (IC if is_gate else 0)
            for i in range(NB):
                transpose_to_xst(0, i, xnat[:, i * CW:i * CW + CW], is_gate)
            for sup, std in ((0, satd), (1, sbtd)):
                m1, m2 = 1 + 2 * sup, 2 + 2 * sup

                def x1_sink(i, pts):
                    for pt, (c0, c1) in zip(pts, chunks):
                        nc.vector.tensor_copy(
                            x1nat[:, i * CW + c0:i * CW + c1], pt[:])
                    transpose_to_xst(m1, i, x1nat[:, i * CW:i * CW + CW],
                                     is_gate)

                spmm(std, xnat, chunks, x1_sink)

                # x2' = S x1 - 0.5 x0  (W of the k=2 terms pre-doubled)
                def x2_sink(i, pts):
                    blk = x2p.tile([128, CW], BF16, tag="x2")
                    for pt, (c0, c1) in zip(pts, chunks):
                        nc.vector.scalar_tensor_tensor(
                            blk[:, c0:c1],
                            xnat[:, i * CW + c0:i * CW + c1],
                            -0.5, pt[:],
                            mybir.AluOpType.mult, mybir.AluOpType.add)
                    transpose_to_xst(m2, i, blk[:, 0:wid], is_gate)

                spmm(std, x1nat, chunks, x2_sink)

        def finalize_inputs():
            for i in range(NB):
                for t in range(3):
                    w = 96 if t < 2 else 64
                    pt = psum.tile([w, 128], BF16, tag="ps", name=f"pfin{t}")
                    nc.tensor.transpose(
                        pt[:], xicat[:, i * 256 + t * 96:i * 256 + t * 96 + w],
                        identb[:])
                    nc.vector.tensor_copy(xpk[t][:w, i * 128:(i + 1) * 128],
                                          pt[:])

        def w_stage(is_gate):
            """Projection + activation. Gate: sigmoid -> r (into x0sb),
            u (to DRAM). Cand: tanh -> c (to DRAM)."""
            ws, wi, O = (wgs, wgi, 128) if is_gate else (wcs, wci, U)
            for b in range(BL):
                t, k = b // 3, b % 3
                for c in range(4):  # n-chunks of 512
                    pt = psum.tile([O, 512], F32, tag="ps")
                    bp = (b % 2) * U
                    for m in range(M):
                        rs = xst_s(m, b // 2)[bp:bp + U, c * 512:(c + 1) * 512]
                        nc.tensor.matmul(pt[:], ws[bp:bp + U, m * O:(m + 1) * O],
                                         rs, start=(m == 0), stop=False)
                    ri = xpk[t][32 * k:32 * k + PK, c * 512:(c + 1) * 512]
                    nc.tensor.matmul(pt[:], wi[32 * k:32 * k + PK, :O], ri,
                                     start=False, stop=True)
                    h = c // 2
                    cols = slice(b * 1024 + 512 * (c % 2),
                                 b * 1024 + 512 * (c % 2) + 512)
                    if is_gate:
                        rb = small.tile([U, 512], F32, tag="rb")
                        nc.scalar.activation(rb[:], pt[:U, :],
                                             mybir.ActivationFunctionType.Sigmoid,
                                             bias=bg[:U, :])
                        ub = small.tile([U, 512], F32, tag="ub")
                        nc.scalar.activation(ub[:], pt[U:128, :],
                                             mybir.ActivationFunctionType.Sigmoid,
                                             bias=bg[U:128, :])
                        nc.sync.dma_start(ubufd[64 * h:64 * h + 64, cols], ub[:])
                        # r^T into x0 state cols (candidate input, in place)
                        rpt = psum.tile([128, 256], F32, tag="ps")
                        for j in range(4):
                            nc.tensor.transpose(
                                rpt[:, j * U:(j + 1) * U],
                                rb[:, j * 128:(j + 1) * 128], ident[:U, :U])
                        xv = x0sb[:].rearrange("p (i c) -> p i c", c=CW)[
                            :, 4 * c:4 * c + 4, b * U:(b + 1) * U]
                        nc.vector.tensor_mul(
                            xv, xv,
                            rpt[:].rearrange("p (i o) -> p i o", o=U))
                    else:
                        cb = small.tile([U, 512], F32, tag="cb")
                        nc.scalar.activation(cb[:], pt[:, :],
                                             mybir.ActivationFunctionType.Tanh,
                                             bias=bc[:])
                        nc.sync.dma_start(cbufd[64 * h:64 * h + 64, cols], cb[:])

        def final():
            """new_state = c + u*(state - c), natural layout, batched per
            (b, half). u/c come back [64, 1024]; state fp32 re-read."""
            stfv = stfd.rearrange("(i p) c -> p i c", p=128)
            outv = outd.rearrange("(i p) c -> p i c", p=128)
            for b in range(BL):
                for h in range(2):
                    i0 = h * 8
                    ut = small.tile([U, 1024], F32, tag="ut")
                    nc.sync.dma_start(
                        ut[:], ubufd[64 * h:64 * h + 64,
                                     b * 1024:(b + 1) * 1024])
                    ct = small.tile([U, 1024], F32, tag="ct")
                    nc.sync.dma_start(
                        ct[:], cbufd[64 * h:64 * h + 64,
                                     b * 1024:(b + 1) * 1024])
                    stt = small.tile([128, 512], F32, tag="stt")
                    nc.sync.dma_start(
                        stt[:].rearrange("p (i c) -> p i c", c=U),
                        stfv[:, i0:i0 + 8, b * U:(b + 1) * U])
                    cpt = psum.tile([128, 512], F32, tag="ps")
                    upt = psum.tile([128, 512], F32, tag="ps")
                    for j in range(8):
                        nc.tensor.transpose(cpt[:, j * U:(j + 1) * U],
                                            ct[:, j * 128:(j + 1) * 128],
                                            ident[:U, :U])
                        nc.tensor.transpose(upt[:, j * U:(j + 1) * U],
                                            ut[:, j * 128:(j + 1) * 128],
                                            ident[:U, :U])
                    # stt = (stt - c) * u + c, all [128, 512], in place
                    nc.vector.tensor_sub(stt[:], stt[:], cpt[:])
                    nc.vector.tensor_mul(stt[:], stt[:], upt[:])
                    nc.vector.tensor_add(stt[:], stt[:], cpt[:])
                    nc.sync.dma_start(
                        outv[:, i0:i0 + 8, b * U:(b + 1) * U],
                        stt[:].rearrange("p (i c) -> p i c", c=U))

        # ---- gate dconv ----
        dconv(x0sb, x1sb, True)
        finalize_inputs()
        w_stage(True)
        # ---- candidate dconv (x0sb is now candX in its state cols) ----
        dconv(x0sb, x1sb, False)
        w_stage(False)
        final()


_NC_CACHE = {}


def _get_nc():
    if "nc" not in _NC_CACHE:
        _NC_CACHE["nc"] = _build_nc()
    return _NC_CACHE["nc"]


def _host_prep(inputs, state, edges1, vals1, edges2, vals2, W_gate, b_gate,
               W_cand, b_cand):
    import ml_dtypes
    BF = ml_dtypes.bfloat16
    inputs = np.asarray(inputs, np.float32)
    state = np.asarray(state, np.float32)

    def densify_T(edges, vals):
        ST = np.zeros((N, N), np.float32)
        np.add.at(ST, (np.asarray(edges[1]).astype(np.int64),
                       np.asarray(edges[0]).astype(np.int64)),
                  np.asarray(vals, np.float32))
        return ST.astype(BF)

    SaT = densify_T(edges1, vals1)
    SbT = densify_T(edges2, vals2)

    def reorder(Wmat):
        Wmat = np.asarray(Wmat, np.float32)
        O = Wmat.shape[1]
        Wm = Wmat.reshape(F, M, O).copy()
        Wm[:, 2, :] *= 2.0
        Wm[:, 4, :] *= 2.0
        # state rows duplicated at partition bases 0 and 64
        Ws = np.ascontiguousarray(Wm[D_IN:].reshape(U, M * O))
        Ws2 = np.concatenate([Ws, Ws], 0)                       # [128, M*O]
        # input rows (m, fi) packed [10, O], replicated at bases 0/32/64
        Wi = np.ascontiguousarray(Wm[:D_IN].transpose(1, 0, 2).reshape(PK, O))
        Wi2 = np.zeros((128, O), np.float32)
        for base in (0, 32, 64):
            Wi2[base:base + PK] = Wi
        return (Ws2.astype(BF), Wi2.astype(BF))

    wgs, wgi = reorder(W_gate)
    wcs, wci = reorder(W_cand)
    bg = np.asarray(b_gate, np.float32).reshape(128, 1)
    bc = np.asarray(b_cand, np.float32).reshape(U, 1)

    in_maps = []
    for c in range(NCORES):
        bsl = slice(c * BL, (c + 1) * BL)
        st_c = state[bsl].reshape(BL, N, U)
        in_c = inputs[bsl].reshape(BL, N, D_IN)
        statef = np.ascontiguousarray(st_c.transpose(1, 0, 2).reshape(N, SC))
        x0 = np.empty((N, CW), np.float32)
        x0[:, :SC] = statef
        x0[:, SC:] = in_c.transpose(1, 0, 2).reshape(N, IC)
        in_maps.append(dict(x0=x0.astype(BF), statef=statef, sat=SaT,
                            sbt=SbT, wgs=wgs, wgi=wgi, wcs=wcs, wci=wci,
                            bg=bg, bc=bc))
    return in_maps


def kernel(**inputs):
    nc = _get_nc()
    in_maps = _host_prep(**inputs)
    res = run_bass_kernel_spmd(nc, in_maps, list(range(NCORES)))
    outs = []
    for c in range(NCORES):
        o = np.asarray(res.results[c]["out"])          # [N, (b, u)]
        outs.append(o.reshape(N, BL, U).transpose(1, 0, 2).reshape(BL, N * U))
    return np.concatenate(outs, 0).astype(np.float32)
